# revision 13
# baseline (speedup 1.0000x reference)
"""HRA-injected linear on 8 Trainium2 NeuronCores.

Math: reference applies r=8 sequential Householder updates to W, then y = x @ W'^T.
Compact WY form (exact): W' = W (I - V U^T), so y = x @ W^T - (x U) (V^T W^T)^T...
computed as: per core, wvn = (-V)^T @ W^T (on device), P^T = U^T x^T per token
tile (col-packed 4-way on the PE array), and the rank-8 correction is ONE extra
K=128 matmul folded into each PSUM accumulation:
  lhsT = stacked P^T partials [128, 128] (4 groups at partition 32j),
  rhs  = wvn replicated at partitions 32j of a zeroed tile (garbage rows x 0).

Per core (8 cores = 4 token-groups x 2 out-feature-groups):
  y_s [2048, 2048] f32 = x_s [2048, 4096] @ W'_s^T

Host marshals per-core shards: x bf16, W^T bf16 in the device tile layout
(wt[p, nb, k, t] = W[nb*512 + t, k*128 + p]), U chunked and -V chunked bf16.
Device: W^T blocks + early x tiles interleaved on one DMA queue; x transposed
via the DMA xbar; PE runs matmuls only, back-to-back.
"""

import numpy as np
import ml_dtypes
from contextlib import ExitStack

import concourse.bacc as bacc
import concourse.mybir as mybir
import concourse.tile as tile
from concourse.bass_utils import run_bass_kernel_spmd

P = 128
D = 4096          # in_features (contraction)
R = 8             # Householder rank
TOK = 8192        # 4*2048 flattened tokens
O = 4096          # out_features
TOK_GROUPS = 4
O_GROUPS = 2
TOK_S = TOK // TOK_GROUPS   # 2048 tokens per core
O_S = O // O_GROUPS         # 2048 out features per core
KT = D // P                 # 32 contraction tiles
MT = TOK_S // P             # 16 token tiles per core
NBW = 512                   # output block width
NB = O_S // NBW             # 4 blocks

F32 = mybir.dt.float32
BF16 = mybir.dt.bfloat16

N_CORES = 8

_NC = None


def _build():
    nc = bacc.Bacc(None, target_bir_lowering=False)
    x_d = nc.declare_dram_parameter("x", [TOK_S, D], BF16, isOutput=False)
    wt_d = nc.declare_dram_parameter("wt", [P, NB, KT, NBW], BF16, isOutput=False)
    u_d = nc.declare_dram_parameter("u", [P, KT, R], BF16, isOutput=False)
    vn_d = nc.declare_dram_parameter("vn", [P, KT, R], BF16, isOutput=False)
    y_d = nc.declare_dram_parameter("out", [TOK_S, O_S], F32, isOutput=True)

    with tile.TileContext(nc) as tc, ExitStack() as ctx:
        const = ctx.enter_context(tc.tile_pool(name="const", bufs=1))
        wt_pool = ctx.enter_context(tc.tile_pool(name="wtp", bufs=1))
        stg = ctx.enter_context(tc.tile_pool(name="stg", bufs=2))
        xt_pool = ctx.enter_context(tc.tile_pool(name="xtp", bufs=4))
        ysb = ctx.enter_context(tc.tile_pool(name="ysb", bufs=4))
        smal = ctx.enter_context(tc.tile_pool(name="smal", bufs=2))
        psum = ctx.enter_context(tc.tile_pool(name="psum", bufs=1, space="PSUM"))

        u_sb = const.tile([P, KT, R], BF16)
        nc.scalar.dma_start(out=u_sb, in_=u_d[:])
        vn_sb = const.tile([P, KT, R], BF16)
        nc.scalar.dma_start(out=vn_sb, in_=vn_d[:])
        # wvn replicated at partition bases {0,32,64,96}; other rows stay zero
        wvn4 = const.tile([P, NB, NBW], BF16)
        nc.vector.memset(wvn4, 0.0)

        # resident W^T: wt[p, nb, k, t] = W[nb*512 + t, k*128 + p]
        wt = wt_pool.tile([P, NB, KT, NBW], BF16)

        def stage_x(m):
            xst = stg.tile([P, D], BF16, tag="st", name="xst")
            nc.gpsimd.dma_start(out=xst, in_=x_d[m * P:(m + 1) * P, :])
            xt = xt_pool.tile([P, KT, P], BF16, tag="xt", name="xt")
            nc.sync.dma_start(out=xt, in_=xst, transpose=True)
            return xt

        early_xt = {}
        # interleave W-block DMAs with early x tiles on the scalar queue
        for nb in range(NB):
            for q in range(4):
                nc.scalar.dma_start(out=wt[:, nb, q * 8:(q + 1) * 8, :],
                                    in_=wt_d[:, nb, q * 8:(q + 1) * 8, :])
            if nb < NB:
                early_xt[nb] = stage_x(nb)

            # wvn[nb] = (-V)^T @ W^T block  [8, 512]
            ps_wv = psum.tile([R, NBW], F32, tag="pwv", bufs=2, name="ps_wv")
            for k in range(KT):
                nc.tensor.matmul(ps_wv, vn_sb[:, k, :], wt[:, nb, k, :],
                                 start=(k == 0), stop=(k == KT - 1))
            wv_sb = smal.tile([R, NBW], BF16, tag="wv", name="wv_sb")
            nc.vector.tensor_copy(out=wv_sb, in_=ps_wv)
            for j in range(4):
                nc.scalar.dma_start(out=wvn4[32 * j:32 * j + R, nb, :], in_=wv_sb)

        for m in range(MT):
            xt = early_xt[m] if m in early_xt else stage_x(m)

            # P^T partials, col-packed 4-way: group j accumulates k = 4g+j
            ps_p = psum.tile([P, P], F32, tag="pp", bufs=2, name="ps_p")
            for g in range(8):
                for j in range(4):
                    k = 4 * g + j
                    nc.tensor.matmul(ps_p[32 * j:32 * j + R, :], u_sb[:, k, :],
                                     xt[:, k, :], start=(g == 0), stop=(g == 7),
                                     tile_position=(0, 32 * j))
            pt4 = smal.tile([P, P], BF16, tag="pt", name="pt4")
            nc.vector.tensor_copy(out=pt4, in_=ps_p)

            for nb in range(NB):
                ps_y = psum.tile([P, NBW], F32, tag=f"py{nb}", bufs=1,
                                 name=f"ps_y{nb}")
                for k in range(KT):
                    nc.tensor.matmul(ps_y, xt[:, k, :], wt[:, nb, k, :],
                                     start=(k == 0), stop=False)
                # rank-8 correction: garbage rows of pt4 hit zero rows of wvn4
                nc.tensor.matmul(ps_y, pt4, wvn4[:, nb, :],
                                 start=False, stop=True)
                y_t = ysb.tile([P, NBW], F32, tag="y", name="y_t")
                nc.vector.tensor_copy(out=y_t, in_=ps_y)
                nc.scalar.dma_start(
                    out=y_d[m * P:(m + 1) * P, nb * NBW:(nb + 1) * NBW], in_=y_t
                )

    nc.compile()
    return nc


def _get_nc():
    global _NC
    if _NC is None:
        _NC = _build()
    return _NC


def _host_prep(hra_u):
    """Normalize u columns and compute V of the compact WY form, in float64."""
    u = hra_u.astype(np.float64)
    u = u / np.linalg.norm(u, axis=0, keepdims=True)        # [D, R]
    v = np.zeros_like(u)
    for k_ in range(R):
        acc = u[:, k_].copy()
        for j in range(k_):
            acc -= v[:, j] * np.dot(u[:, j], u[:, k_])
        v[:, k_] = 2.0 * acc

    def chunk(m):
        return np.ascontiguousarray(
            m.reshape(KT, P, R).transpose(1, 0, 2)
        ).astype(ml_dtypes.bfloat16)

    return chunk(u), chunk(-v)


def _make_in_maps(x, weight, hra_u):
    u_c, vn = _host_prep(hra_u)
    xf = np.ascontiguousarray(x.reshape(TOK, D)).astype(ml_dtypes.bfloat16)
    wf = weight.astype(ml_dtypes.bfloat16)

    wts = []
    for b in range(O_GROUPS):
        ws = wf[b * O_S:(b + 1) * O_S]                     # [O_S, D]
        # wt[p, nb, k, t] = ws[nb*512 + t, k*128 + p]
        wt = np.ascontiguousarray(
            ws.reshape(NB, NBW, KT, P).transpose(3, 0, 2, 1)
        )
        wts.append(wt)

    in_maps = []
    for core in range(N_CORES):
        a, b = core // O_GROUPS, core % O_GROUPS
        in_maps.append({
            "x": np.ascontiguousarray(xf[a * TOK_S:(a + 1) * TOK_S]),
            "wt": wts[b],
            "u": u_c,
            "vn": vn,
        })
    return in_maps


def kernel(x, weight, hra_u):
    nc = _get_nc()
    in_maps = _make_in_maps(x, weight, hra_u)
    res = run_bass_kernel_spmd(nc, in_maps, core_ids=list(range(N_CORES))).results

    y = np.empty((TOK, O), dtype=np.float32)
    for core in range(N_CORES):
        a, b = core // O_GROUPS, core % O_GROUPS
        y[a * TOK_S:(a + 1) * TOK_S, b * O_S:(b + 1) * O_S] = res[core]["out"]
    return y.reshape(x.shape[0], x.shape[1], O)


# revision 14
# speedup vs baseline: 1.0276x; 1.0276x over previous
"""HRA-injected linear on 8 Trainium2 NeuronCores.

Math: reference applies r=8 sequential Householder updates to W, then y = x @ W'^T.
Compact WY form (exact): W' = W (I - V U^T), so y = x @ W^T - (x U) (V^T W^T)^T...
computed as: per core, wvn = (-V)^T @ W^T (on device), P^T = U^T x^T per token
tile (col-packed 4-way on the PE array), and the rank-8 correction is ONE extra
K=128 matmul folded into each PSUM accumulation:
  lhsT = stacked P^T partials [128, 128] (4 groups at partition 32j),
  rhs  = wvn replicated at partitions 32j of a zeroed tile (garbage rows x 0).

Per core (8 cores = 4 token-groups x 2 out-feature-groups):
  y_s [2048, 2048] f32 = x_s [2048, 4096] @ W'_s^T

Host marshals per-core shards: x bf16, W^T bf16 in the device tile layout
(wt[p, nb, k, t] = W[nb*512 + t, k*128 + p]), U chunked and -V chunked bf16.
Device: W^T blocks + early x tiles interleaved on one DMA queue; x transposed
via the DMA xbar; PE runs matmuls only, back-to-back.
"""

import numpy as np
import ml_dtypes
from contextlib import ExitStack

import concourse.bacc as bacc
import concourse.mybir as mybir
import concourse.tile as tile
from concourse.bass_utils import run_bass_kernel_spmd

P = 128
D = 4096          # in_features (contraction)
R = 8             # Householder rank
TOK = 8192        # 4*2048 flattened tokens
O = 4096          # out_features
TOK_GROUPS = 4
O_GROUPS = 2
TOK_S = TOK // TOK_GROUPS   # 2048 tokens per core
O_S = O // O_GROUPS         # 2048 out features per core
KT = D // P                 # 32 contraction tiles
MT = TOK_S // P             # 16 token tiles per core
NBW = 512                   # output block width
NB = O_S // NBW             # 4 blocks

F32 = mybir.dt.float32
BF16 = mybir.dt.bfloat16

N_CORES = 8

_NC = None


def _build():
    nc = bacc.Bacc(None, target_bir_lowering=False)
    x_d = nc.declare_dram_parameter("x", [TOK_S, D], BF16, isOutput=False)
    wt_d = nc.declare_dram_parameter("wt", [P, NB, KT, NBW], BF16, isOutput=False)
    u_d = nc.declare_dram_parameter("u", [P, KT, R], BF16, isOutput=False)
    vn_d = nc.declare_dram_parameter("vn", [P, KT, R], BF16, isOutput=False)
    y_d = nc.declare_dram_parameter("out", [TOK_S, O_S], F32, isOutput=True)

    with tile.TileContext(nc) as tc, ExitStack() as ctx:
        const = ctx.enter_context(tc.tile_pool(name="const", bufs=1))
        wt_pool = ctx.enter_context(tc.tile_pool(name="wtp", bufs=1))
        stg = ctx.enter_context(tc.tile_pool(name="stg", bufs=2))
        xt_pool = ctx.enter_context(tc.tile_pool(name="xtp", bufs=3))
        ysb = ctx.enter_context(tc.tile_pool(name="ysb", bufs=4))
        smal = ctx.enter_context(tc.tile_pool(name="smal", bufs=2))
        psum = ctx.enter_context(tc.tile_pool(name="psum", bufs=1, space="PSUM"))

        u_sb = const.tile([P, KT, R], BF16)
        nc.scalar.dma_start(out=u_sb, in_=u_d[:])
        vn_sb = const.tile([P, KT, R], BF16)
        nc.scalar.dma_start(out=vn_sb, in_=vn_d[:])
        # wvn replicated at partition bases {0,32,64,96}; other rows stay zero
        wvn4 = const.tile([P, NB, NBW], BF16)
        nc.vector.memset(wvn4, 0.0)

        # resident W^T: wt[p, nb, k, t] = W[nb*512 + t, k*128 + p]
        wt = wt_pool.tile([P, NB, KT, NBW], BF16)

        def stage_x(m):
            xst = stg.tile([P, D], BF16, tag="st", name="xst")
            nc.scalar.dma_start(out=xst, in_=x_d[m * P:(m + 1) * P, :])
            xt = xt_pool.tile([P, KT, P], BF16, tag="xt", name="xt")
            nc.sync.dma_start(out=xt, in_=xst, transpose=True)
            return xt

        early_xt = {}
        # interleave W-block DMAs with early x tiles on the scalar queue
        for nb in range(NB):
            for q in range(4):
                nc.scalar.dma_start(out=wt[:, nb, q * 8:(q + 1) * 8, :],
                                    in_=wt_d[:, nb, q * 8:(q + 1) * 8, :])
            if nb < 3:
                early_xt[nb] = stage_x(nb)

            # wvn[nb] = (-V)^T @ W^T block  [8, 512]
            ps_wv = psum.tile([R, NBW], F32, tag="pwv", bufs=2, name="ps_wv")
            for k in range(KT):
                nc.tensor.matmul(ps_wv, vn_sb[:, k, :], wt[:, nb, k, :],
                                 start=(k == 0), stop=(k == KT - 1))
            wv_sb = smal.tile([R, NBW], BF16, tag="wv", name="wv_sb")
            nc.vector.tensor_copy(out=wv_sb, in_=ps_wv)
            for j in range(4):
                nc.scalar.dma_start(out=wvn4[32 * j:32 * j + R, nb, :], in_=wv_sb)

        for m in range(MT):
            xt = early_xt[m] if m in early_xt else stage_x(m)

            # P^T partials, col-packed 4-way: group j accumulates k = 4g+j
            ps_p = psum.tile([P, P], F32, tag="pp", bufs=2, name="ps_p")
            for g in range(8):
                for j in range(4):
                    k = 4 * g + j
                    nc.tensor.matmul(ps_p[32 * j:32 * j + R, :], u_sb[:, k, :],
                                     xt[:, k, :], start=(g == 0), stop=(g == 7),
                                     tile_position=(0, 32 * j))
            pt4 = smal.tile([P, P], BF16, tag="pt", name="pt4")
            nc.vector.tensor_copy(out=pt4, in_=ps_p)

            for nb in range(NB):
                ps_y = psum.tile([P, NBW], F32, tag=f"py{nb}", bufs=1,
                                 name=f"ps_y{nb}")
                for k in range(KT):
                    nc.tensor.matmul(ps_y, xt[:, k, :], wt[:, nb, k, :],
                                     start=(k == 0), stop=False)
                # rank-8 correction: garbage rows of pt4 hit zero rows of wvn4
                nc.tensor.matmul(ps_y, pt4, wvn4[:, nb, :],
                                 start=False, stop=True)
                y_t = ysb.tile([P, NBW], F32, tag="y", name="y_t")
                nc.vector.tensor_copy(out=y_t, in_=ps_y)
                nc.scalar.dma_start(
                    out=y_d[m * P:(m + 1) * P, nb * NBW:(nb + 1) * NBW], in_=y_t
                )

    nc.compile()
    return nc


def _get_nc():
    global _NC
    if _NC is None:
        _NC = _build()
    return _NC


def _host_prep(hra_u):
    """Normalize u columns and compute V of the compact WY form, in float64."""
    u = hra_u.astype(np.float64)
    u = u / np.linalg.norm(u, axis=0, keepdims=True)        # [D, R]
    v = np.zeros_like(u)
    for k_ in range(R):
        acc = u[:, k_].copy()
        for j in range(k_):
            acc -= v[:, j] * np.dot(u[:, j], u[:, k_])
        v[:, k_] = 2.0 * acc

    def chunk(m):
        return np.ascontiguousarray(
            m.reshape(KT, P, R).transpose(1, 0, 2)
        ).astype(ml_dtypes.bfloat16)

    return chunk(u), chunk(-v)


def _make_in_maps(x, weight, hra_u):
    u_c, vn = _host_prep(hra_u)
    xf = np.ascontiguousarray(x.reshape(TOK, D)).astype(ml_dtypes.bfloat16)
    wf = weight.astype(ml_dtypes.bfloat16)

    wts = []
    for b in range(O_GROUPS):
        ws = wf[b * O_S:(b + 1) * O_S]                     # [O_S, D]
        # wt[p, nb, k, t] = ws[nb*512 + t, k*128 + p]
        wt = np.ascontiguousarray(
            ws.reshape(NB, NBW, KT, P).transpose(3, 0, 2, 1)
        )
        wts.append(wt)

    in_maps = []
    for core in range(N_CORES):
        a, b = core // O_GROUPS, core % O_GROUPS
        in_maps.append({
            "x": np.ascontiguousarray(xf[a * TOK_S:(a + 1) * TOK_S]),
            "wt": wts[b],
            "u": u_c,
            "vn": vn,
        })
    return in_maps


def kernel(x, weight, hra_u):
    nc = _get_nc()
    in_maps = _make_in_maps(x, weight, hra_u)
    res = run_bass_kernel_spmd(nc, in_maps, core_ids=list(range(N_CORES))).results

    y = np.empty((TOK, O), dtype=np.float32)
    for core in range(N_CORES):
        a, b = core // O_GROUPS, core % O_GROUPS
        y[a * TOK_S:(a + 1) * TOK_S, b * O_S:(b + 1) * O_S] = res[core]["out"]
    return y.reshape(x.shape[0], x.shape[1], O)


# revision 15
# speedup vs baseline: 1.0390x; 1.0111x over previous
"""HRA-injected linear on 8 Trainium2 NeuronCores.

Math: reference applies r=8 sequential Householder updates to W, then y = x @ W'^T.
Compact WY form (exact): W' = W (I - V U^T), so y = x @ W^T - (x U) (V^T W^T)^T...
computed as: per core, wvn = (-V)^T @ W^T (on device), P^T = U^T x^T per token
tile (col-packed 4-way on the PE array), and the rank-8 correction is ONE extra
K=128 matmul folded into each PSUM accumulation:
  lhsT = stacked P^T partials [128, 128] (4 groups at partition 32j),
  rhs  = wvn replicated at partitions 32j of a zeroed tile (garbage rows x 0).

Per core (8 cores = 4 token-groups x 2 out-feature-groups):
  y_s [2048, 2048] f32 = x_s [2048, 4096] @ W'_s^T

Host marshals per-core shards: x bf16, W^T bf16 in the device tile layout
(wt[p, nb, k, t] = W[nb*512 + t, k*128 + p]), U chunked and -V chunked bf16.
Device: W^T blocks + early x tiles interleaved on one DMA queue; x transposed
via the DMA xbar; PE runs matmuls only, back-to-back.
"""

import numpy as np
import ml_dtypes
from contextlib import ExitStack

import concourse.bacc as bacc
import concourse.mybir as mybir
import concourse.tile as tile
from concourse.bass_utils import run_bass_kernel_spmd

P = 128
D = 4096          # in_features (contraction)
R = 8             # Householder rank
TOK = 8192        # 4*2048 flattened tokens
O = 4096          # out_features
TOK_GROUPS = 4
O_GROUPS = 2
TOK_S = TOK // TOK_GROUPS   # 2048 tokens per core
O_S = O // O_GROUPS         # 2048 out features per core
KT = D // P                 # 32 contraction tiles
MT = TOK_S // P             # 16 token tiles per core
NBW = 512                   # output block width
NB = O_S // NBW             # 4 blocks

F32 = mybir.dt.float32
BF16 = mybir.dt.bfloat16

N_CORES = 8

_NC = None


def _build():
    nc = bacc.Bacc(None, target_bir_lowering=False)
    x_d = nc.declare_dram_parameter("x", [TOK_S, D], BF16, isOutput=False)
    wt_d = nc.declare_dram_parameter("wt", [P, NB, KT, NBW], BF16, isOutput=False)
    u_d = nc.declare_dram_parameter("u", [P, KT, R], BF16, isOutput=False)
    vn_d = nc.declare_dram_parameter("vn", [P, KT, R], BF16, isOutput=False)
    y_d = nc.declare_dram_parameter("out", [TOK_S, O_S], F32, isOutput=True)

    with tile.TileContext(nc) as tc, ExitStack() as ctx:
        const = ctx.enter_context(tc.tile_pool(name="const", bufs=1))
        wt_pool = ctx.enter_context(tc.tile_pool(name="wtp", bufs=1))
        stg = ctx.enter_context(tc.tile_pool(name="stg", bufs=2))
        xt_pool = ctx.enter_context(tc.tile_pool(name="xtp", bufs=3))
        ysb = ctx.enter_context(tc.tile_pool(name="ysb", bufs=4))
        smal = ctx.enter_context(tc.tile_pool(name="smal", bufs=2))
        psum = ctx.enter_context(tc.tile_pool(name="psum", bufs=1, space="PSUM"))

        u_sb = const.tile([P, KT, R], BF16)
        nc.scalar.dma_start(out=u_sb, in_=u_d[:])
        vn_sb = const.tile([P, KT, R], BF16)
        nc.scalar.dma_start(out=vn_sb, in_=vn_d[:])
        # wvn replicated at partition bases {0,32,64,96}; other rows stay zero
        wvn4 = const.tile([P, NB, NBW], BF16)
        nc.vector.memset(wvn4, 0.0)

        # resident W^T: wt[p, nb, k, t] = W[nb*512 + t, k*128 + p]
        wt = wt_pool.tile([P, NB, KT, NBW], BF16)

        def stage_x(m):
            xt = xt_pool.tile([P, KT, P], BF16, tag="xt", name="xt")
            if m < 3:
                # prologue tiles: DRAM-direct xbar on the idle sync queue,
                # keeping the scalar queue free for W blocks
                nc.sync.dma_start(out=xt, in_=x_d[m * P:(m + 1) * P, :],
                                  transpose=True)
            else:
                xst = stg.tile([P, D], BF16, tag="st", name="xst")
                nc.scalar.dma_start(out=xst, in_=x_d[m * P:(m + 1) * P, :])
                nc.sync.dma_start(out=xt, in_=xst, transpose=True)
            return xt

        early_xt = {}
        # interleave W-block DMAs with early x tiles on the scalar queue
        for nb in range(NB):
            for q in range(4):
                nc.scalar.dma_start(out=wt[:, nb, q * 8:(q + 1) * 8, :],
                                    in_=wt_d[:, nb, q * 8:(q + 1) * 8, :])
            if nb < 3:
                early_xt[nb] = stage_x(nb)

            # wvn[nb] = (-V)^T @ W^T block  [8, 512]
            ps_wv = psum.tile([R, NBW], F32, tag="pwv", bufs=2, name="ps_wv")
            for k in range(KT):
                nc.tensor.matmul(ps_wv, vn_sb[:, k, :], wt[:, nb, k, :],
                                 start=(k == 0), stop=(k == KT - 1))
            wv_sb = smal.tile([R, NBW], BF16, tag="wv", name="wv_sb")
            nc.vector.tensor_copy(out=wv_sb, in_=ps_wv)
            for j in range(4):
                nc.scalar.dma_start(out=wvn4[32 * j:32 * j + R, nb, :], in_=wv_sb)

        for m in range(MT):
            xt = early_xt[m] if m in early_xt else stage_x(m)

            # P^T partials, col-packed 4-way: group j accumulates k = 4g+j
            ps_p = psum.tile([P, P], F32, tag="pp", bufs=2, name="ps_p")
            for g in range(8):
                for j in range(4):
                    k = 4 * g + j
                    nc.tensor.matmul(ps_p[32 * j:32 * j + R, :], u_sb[:, k, :],
                                     xt[:, k, :], start=(g == 0), stop=(g == 7),
                                     tile_position=(0, 32 * j))
            pt4 = smal.tile([P, P], BF16, tag="pt", name="pt4")
            nc.vector.tensor_copy(out=pt4, in_=ps_p)

            for nb in range(NB):
                ps_y = psum.tile([P, NBW], F32, tag=f"py{nb}", bufs=1,
                                 name=f"ps_y{nb}")
                for k in range(KT):
                    nc.tensor.matmul(ps_y, xt[:, k, :], wt[:, nb, k, :],
                                     start=(k == 0), stop=False)
                # rank-8 correction: garbage rows of pt4 hit zero rows of wvn4
                nc.tensor.matmul(ps_y, pt4, wvn4[:, nb, :],
                                 start=False, stop=True)
                y_t = ysb.tile([P, NBW], F32, tag="y", name="y_t")
                nc.vector.tensor_copy(out=y_t, in_=ps_y)
                nc.scalar.dma_start(
                    out=y_d[m * P:(m + 1) * P, nb * NBW:(nb + 1) * NBW], in_=y_t
                )

    nc.compile()
    return nc


def _get_nc():
    global _NC
    if _NC is None:
        _NC = _build()
    return _NC


def _host_prep(hra_u):
    """Normalize u columns and compute V of the compact WY form, in float64."""
    u = hra_u.astype(np.float64)
    u = u / np.linalg.norm(u, axis=0, keepdims=True)        # [D, R]
    v = np.zeros_like(u)
    for k_ in range(R):
        acc = u[:, k_].copy()
        for j in range(k_):
            acc -= v[:, j] * np.dot(u[:, j], u[:, k_])
        v[:, k_] = 2.0 * acc

    def chunk(m):
        return np.ascontiguousarray(
            m.reshape(KT, P, R).transpose(1, 0, 2)
        ).astype(ml_dtypes.bfloat16)

    return chunk(u), chunk(-v)


def _make_in_maps(x, weight, hra_u):
    u_c, vn = _host_prep(hra_u)
    xf = np.ascontiguousarray(x.reshape(TOK, D)).astype(ml_dtypes.bfloat16)
    wf = weight.astype(ml_dtypes.bfloat16)

    wts = []
    for b in range(O_GROUPS):
        ws = wf[b * O_S:(b + 1) * O_S]                     # [O_S, D]
        # wt[p, nb, k, t] = ws[nb*512 + t, k*128 + p]
        wt = np.ascontiguousarray(
            ws.reshape(NB, NBW, KT, P).transpose(3, 0, 2, 1)
        )
        wts.append(wt)

    in_maps = []
    for core in range(N_CORES):
        a, b = core // O_GROUPS, core % O_GROUPS
        in_maps.append({
            "x": np.ascontiguousarray(xf[a * TOK_S:(a + 1) * TOK_S]),
            "wt": wts[b],
            "u": u_c,
            "vn": vn,
        })
    return in_maps


def kernel(x, weight, hra_u):
    nc = _get_nc()
    in_maps = _make_in_maps(x, weight, hra_u)
    res = run_bass_kernel_spmd(nc, in_maps, core_ids=list(range(N_CORES))).results

    y = np.empty((TOK, O), dtype=np.float32)
    for core in range(N_CORES):
        a, b = core // O_GROUPS, core % O_GROUPS
        y[a * TOK_S:(a + 1) * TOK_S, b * O_S:(b + 1) * O_S] = res[core]["out"]
    return y.reshape(x.shape[0], x.shape[1], O)
